# revision 1
# baseline (speedup 1.0000x reference)
import sys
sys.path.insert(0, '/opt/trn_rl_repo')
import math
import numpy as np
import ml_dtypes

import concourse.bass as bass
import concourse.tile as tile
from concourse import bacc, mybir
from concourse.bass_utils import run_bass_kernel_spmd
from concourse.masks import make_identity

DIM = 2048
NUM_HEADS = 32
NUM_KV_HEADS = 8
HD = 64
BSZ, SEQ = 2, 2048
THRESHOLD = 0.05
S = SEQ                      # seq per core (batch-sharded)
HPC = 8                      # q heads per core
KVPC = 2                     # kv heads per core
NPAIR = 4                    # head pairs per core
SB = 512                     # s-block
NSB = S // SB                # 4
NDC = DIM // 128             # 16 contraction chunks
NQT = S // 128               # 16 q tiles
NQB = S // SB                # 4 q blocks

f32 = mybir.dt.float32
f32r = mybir.dt.float32r
bf16 = mybir.dt.bfloat16
bf = ml_dtypes.bfloat16
EXP = mybir.ActivationFunctionType.Exp
AX = mybir.AxisListType.X
MAX = mybir.AluOpType.max
MIN = mybir.AluOpType.min


def _ternarize(w):
    w = w.astype(np.float64)
    scale = max(np.abs(w).mean(), 1e-6)
    return np.where(w > THRESHOLD * scale, 1.0,
                    np.where(w < -THRESHOLD * scale, -1.0, 0.0))


def _split_hi_lo(x32):
    hi = x32.astype(bf)
    lo = (x32 - hi.astype(np.float32)).astype(bf)
    return hi, lo


DEBUG = False


def build_program():
    nc = bacc.Bacc(None, target_bir_lowering=False, debug=False)

    def din(name, shape, dt):
        return nc.dram_tensor(name, list(shape), dt, kind="ExternalInput").ap()

    xhi_d = din("xhi", (DIM, S), bf16)       # x[b].T hi
    xlo_d = din("xlo", (DIM, S), bf16)       # x[b].T lo
    wq_d = din("wq", (DIM, 512), bf16)       # ternary(wq).T/8 cols of 8 heads
    wk_d = din("wk", (DIM, 128), bf16)       # ternary(wk).T cols [k0|k1]
    wv_d = din("wv", (DIM, 128), bf16)       # ternary(wv).T cols [v0|v1]
    wo_d = din("wo", (512, DIM), f32r)       # ternary(wo).T rows = core feats
    tri_d = din("tri", (128, 128), bf16)     # lower-tri 0/1 mask
    oT_d = nc.dram_tensor("oT", [DIM, S], f32, kind="ExternalOutput").ap()
    dbg = {}
    if DEBUG:
        for nm, shape in [("d_qthi", (128, S)), ("d_qtlo", (128, S)),
                          ("d_kkhi", (128, S)), ("d_kklo", (128, S)),
                          ("d_va", (128, NDC * 65)), ("d_ot", (128, S)),
                          ("d_p", (128, 4 * S)), ("d_pt", (128, NDC * SB))]:
            dbg[nm] = nc.dram_tensor(nm, list(shape), f32, kind="ExternalOutput").ap()
    scr_d = nc.dram_tensor("scr", [HPC, NQB, SB], f32).ap()  # recip rows

    with tile.TileContext(nc) as tc:
        # ---------------- persistent tiles ----------------
        with tc.tile_pool(name="persist", bufs=1) as pp:
            wq_sb = pp.tile([128, NDC, 512], bf16)
            wk_sb = pp.tile([128, NDC, 128], bf16)
            wv_sb = pp.tile([128, NDC, 128], bf16)
            tri = pp.tile([128, 128], bf16)
            nc.sync.dma_start(tri[:], tri_d[:])
            identb = pp.tile([128, 128], bf16)
            make_identity(nc, identb[:])

            # projection results
            qt_hi = [pp.tile([128, S], bf16, tag=f"qth{m}", name=f"qth{m}") for m in range(NPAIR)]
            qt_lo = [pp.tile([128, S], bf16, tag=f"qtl{m}", name=f"qtl{m}") for m in range(NPAIR)]
            kk_hi = [pp.tile([128, S], bf16, tag=f"kkh{v}", name=f"kkh{v}") for v in range(KVPC)]
            kk_lo = [pp.tile([128, S], bf16, tag=f"kkl{v}", name=f"kkl{v}") for v in range(KVPC)]
            va = [pp.tile([128, NDC, 65], bf16, tag=f"va{v}", name=f"va{v}") for v in range(KVPC)]
            ot = [pp.tile([128, S], f32r, tag=f"ot{m}", name=f"ot{m}") for m in range(NPAIR)]
            for v in range(KVPC):
                nc.vector.memset(va[v][:, :, 64:65], 1.0)

            # ---------------- phase 1: projections ----------------
            with tc.tile_pool(name="xp", bufs=4) as xp, \
                 tc.tile_pool(name="evac", bufs=2) as ev, \
                 tc.tile_pool(name="psp", bufs=1, space="PSUM") as psp:
                for sb_i in range(NSB):
                    ssl = bass.ts(sb_i, SB)
                    ps_q = [psp.tile([128, SB], f32, tag=f"psq{m}", name=f"psq{m}") for m in range(NPAIR)]
                    ps_k = psp.tile([128, SB], f32, tag="psk")
                    ps_v = psp.tile([128, SB], f32, tag="psv")
                    for dc in range(NDC):
                        xhi = xp.tile([128, SB], bf16, tag="xhi")
                        xlo = xp.tile([128, SB], bf16, tag="xlo")
                        nc.sync.dma_start(xhi[:], xhi_d[dc * 128:(dc + 1) * 128, ssl])
                        nc.sync.dma_start(xlo[:], xlo_d[dc * 128:(dc + 1) * 128, ssl])
                        if sb_i == 0:
                            nc.sync.dma_start(wq_sb[:, dc, :], wq_d[dc * 128:(dc + 1) * 128, :])
                            nc.sync.dma_start(wk_sb[:, dc, :], wk_d[dc * 128:(dc + 1) * 128, :])
                            nc.sync.dma_start(wv_sb[:, dc, :], wv_d[dc * 128:(dc + 1) * 128, :])
                        st = (dc == 0)
                        sp = (dc == NDC - 1)
                        for m in range(NPAIR):
                            wsl = wq_sb[:, dc, bass.ts(m, 128)]
                            nc.tensor.matmul(ps_q[m][:], wsl, xhi[:], start=st, stop=False)
                            nc.tensor.matmul(ps_q[m][:], wsl, xlo[:], start=False, stop=sp)
                        nc.tensor.matmul(ps_k[:], wk_sb[:, dc, :], xhi[:], start=st, stop=False)
                        nc.tensor.matmul(ps_k[:], wk_sb[:, dc, :], xlo[:], start=False, stop=sp)
                        for j in range(4):
                            nc.tensor.matmul(ps_v[:, bass.ts(j, 128)],
                                             xhi[:, bass.ts(j, 128)],
                                             wv_sb[:, dc, :],
                                             start=(st and j == 0), stop=sp)
                    # evacuate Q (hi/lo split)
                    for m in range(NPAIR):
                        nc.scalar.copy(qt_hi[m][:, ssl], ps_q[m][:])
                        nc.vector.tensor_sub(qt_lo[m][:, ssl], ps_q[m][:], qt_hi[m][:, ssl])
                    # evacuate K (hi/lo + duplicate rows for row-packing)
                    k_hi = ev.tile([128, SB], bf16, tag="khi")
                    k_lo = ev.tile([128, SB], bf16, tag="klo")
                    nc.scalar.copy(k_hi[:], ps_k[:])
                    nc.vector.tensor_sub(k_lo[:], ps_k[:], k_hi[:])
                    for v in range(KVPC):
                        vs = bass.ds(v * 64, 64)
                        nc.sync.dma_start(kk_hi[v][0:64, ssl], k_hi[vs, :])
                        nc.sync.dma_start(kk_hi[v][64:128, ssl], k_hi[vs, :])
                        nc.sync.dma_start(kk_lo[v][0:64, ssl], k_lo[vs, :])
                        nc.sync.dma_start(kk_lo[v][64:128, ssl], k_lo[vs, :])
                    for j in range(4):
                        ch = sb_i * 4 + j
                        nc.scalar.copy(va[0][:, ch, 0:64], ps_v[:, bass.ds(j * 128, 64)])
                        nc.scalar.copy(va[1][:, ch, 0:64], ps_v[:, bass.ds(j * 128 + 64, 64)])

            # ---------------- phase 2: attention ----------------
            with tc.tile_pool(name="att", bufs=1) as ap, \
                 tc.tile_pool(name="stat", bufs=4) as stp, \
                 tc.tile_pool(name="ps2", bufs=1, space="PSUM") as ps2:
                p_t = [ap.tile([128, 4, S], bf16, tag=f"p{h}", name=f"p{h}") for h in range(2)]
                pt_t = [ap.tile([128, NDC, SB], bf16, tag=f"pt{h}", name=f"pt{h}") for h in range(2)]
                for hp in range(NPAIR):
                    kv = hp // 2
                    for qb in range(NQB):
                        nch = 4 * (qb + 1)
                        # -- scores + exp per q tile --
                        for j in range(4):
                            qi = qb * 4 + j
                            nk = qi // 4 + 1
                            qsl = bass.ts(qi, 128)
                            nmx = [stp.tile([128, 4], f32, tag=f"nmx{h}", name=f"nmx{h}") for h in range(2)]
                            for kb in range(nk):
                                kw = 512 if kb < nk - 1 else 128 * (qi % 4 + 1)
                                ksl = bass.ds(kb * 512, kw)
                                s0 = [ps2.tile([128, 512], f32, tag=f"s0{h}", bufs=1, name=f"s0{h}") for h in range(2)]
                                for h in range(2):
                                    nc.tensor.matmul(
                                        s0[h][:, 0:kw],
                                        qt_hi[hp][bass.ds(h * 64, 64), qsl],
                                        kk_hi[kv][bass.ds(h * 64, 64), ksl],
                                        start=True, stop=True,
                                        tile_position=(h * 64, 0))
                                    if kb == nk - 1:
                                        nc.vector.tensor_add(
                                            s0[h][:, kw - 128:kw],
                                            s0[h][:, kw - 128:kw], tri[:])
                                    nc.vector.tensor_reduce(
                                        nmx[h][:, kb:kb + 1], s0[h][:, 0:kw],
                                        AX, MAX, negate=True)
                            negmax = [stp.tile([128, 1], f32, tag=f"ngm{h}", name=f"ngm{h}") for h in range(2)]
                            for h in range(2):
                                nc.vector.tensor_reduce(
                                    negmax[h][:], nmx[h][:, 0:nk], AX, MIN)
                            # accurate scores (hh + hl + lh) then exp
                            for kb in range(nk):
                                kw = 512 if kb < nk - 1 else 128 * (qi % 4 + 1)
                                ksl = bass.ds(kb * 512, kw)
                                sa = [ps2.tile([128, 512], f32, tag=f"sa{h}", name=f"sa{h}") for h in range(2)]
                                for h in range(2):
                                    hs = bass.ds(h * 64, 64)
                                    tp = (h * 64, 0)
                                    nc.tensor.matmul(sa[h][:, 0:kw], qt_hi[hp][hs, qsl],
                                                     kk_hi[kv][hs, ksl], start=True,
                                                     stop=False, tile_position=tp)
                                    nc.tensor.matmul(sa[h][:, 0:kw], qt_hi[hp][hs, qsl],
                                                     kk_lo[kv][hs, ksl], start=False,
                                                     stop=False, tile_position=tp)
                                    nc.tensor.matmul(sa[h][:, 0:kw], qt_lo[hp][hs, qsl],
                                                     kk_hi[kv][hs, ksl], start=False,
                                                     stop=True, tile_position=tp)
                                    if kb == nk - 1:
                                        nc.vector.tensor_add(
                                            sa[h][:, kw - 128:kw],
                                            sa[h][:, kw - 128:kw], tri[:])
                                    nc.scalar.activation(
                                        p_t[h][:, j, ksl], sa[h][:, 0:kw], EXP,
                                        bias=negmax[h][:], scale=1.0)
                        # -- transpose P, zero invalid chunks --
                        for h in range(2):
                            for c in range(nch):
                                jlo = max(0, c - 4 * qb)   # first valid q strip
                                if jlo > 0:
                                    nc.gpsimd.memset(
                                        pt_t[h][:, c, 0:jlo * 128], 0.0)
                                if jlo > 3:
                                    continue
                                tps = ps2.tile([128, 512], bf16, tag="tp",
                                               bufs=2, name=f"tp{h}")
                                for j in range(jlo, 4):
                                    nc.tensor.matmul(
                                        tps[:, bass.ts(j, 128)],
                                        p_t[h][:, j, bass.ts(c, 128)],
                                        identb[:], is_transpose=True,
                                        start=(j == jlo), stop=(j == 3))
                                cp = nc.vector.tensor_copy if (c % 3 == 0) else nc.scalar.copy
                                cp(pt_t[h][:, c, bass.ds(jlo * 128, (4 - jlo) * 128)],
                                   tps[:, bass.ds(jlo * 128, (4 - jlo) * 128)])
                        # -- PV + normalize --
                        for h in range(2):
                            hg = hp * 2 + h
                            pv = ps2.tile([65, 512], f32, tag=f"pv{h}")
                            for c in range(nch):
                                nc.tensor.matmul(pv[:], va[kv][:, c, :], pt_t[h][:, c, :],
                                                 start=(c == 0), stop=(c == nch - 1))
                            rr = stp.tile([1, 512], f32, tag=f"rr{h}")
                            nc.vector.reciprocal(rr[:], pv[64:65, :])
                            nc.sync.dma_start(scr_d[hg, qb, :], rr[:])
                            bc = stp.tile([64, 512], f32, tag=f"bc{h}")
                            nc.sync.dma_start(
                                bc[:], scr_d[hg:hg + 1, qb, :].to_broadcast((64, 512)))
                            nc.vector.tensor_mul(
                                ot[hp][bass.ds(h * 64, 64), bass.ts(qb, 512)],
                                pv[0:64, :], bc[:])
                if DEBUG and hp == 0:
                    dt1 = ap.tile([128, 4, S], f32, tag="dbg3", name="cpp")
                    nc.vector.tensor_copy(dt1[:], p_t[0][:])
                    nc.sync.dma_start(dbg["d_p"][:], dt1[:, :, :].rearrange("p a b -> p (a b)"))
                    dt2 = ap.tile([128, NDC, SB], f32, tag="dbg4", name="cppt")
                    nc.vector.tensor_copy(dt2[:], pt_t[0][:])
                    nc.sync.dma_start(dbg["d_pt"][:], dt2[:, :, :].rearrange("p a b -> p (a b)"))

            if DEBUG:
                with tc.tile_pool(name="dbgq", bufs=1) as dq:
                    dt3 = dq.tile([128, S], f32, tag="dbg5", name="cpot")
                    nc.vector.tensor_copy(dt3[:], ot[0][:])
                    nc.sync.dma_start(dbg["d_ot"][:], dt3[:])

            # ---------------- phase 3: output projection ----------------
            with tc.tile_pool(name="wop", bufs=1) as wp, \
                 tc.tile_pool(name="op", bufs=3) as op, \
                 tc.tile_pool(name="ps3", bufs=2, space="PSUM") as ps3:
                wo_sb = wp.tile([128, 4, DIM], f32r)
                for fc in range(4):
                    nc.sync.dma_start(wo_sb[:, fc, :], wo_d[fc * 128:(fc + 1) * 128, :])
                for m in range(16):
                    for sb_i in range(NSB):
                        ps_o = ps3.tile([128, SB], f32, tag="pso")
                        for fc in range(4):
                            nc.tensor.matmul(
                                ps_o[:], wo_sb[:, fc, bass.ts(m, 128)],
                                ot[fc][:, bass.ts(sb_i, SB)],
                                start=(fc == 0), stop=(fc == 3))
                        osb = op.tile([128, SB], f32, tag="osb")
                        nc.scalar.copy(osb[:], ps_o[:])
                        nc.sync.dma_start(
                            oT_d[bass.ts(m, 128), bass.ts(sb_i, SB)], osb[:])

    nc.compile()
    return nc


_PROG = None


def kernel(x, wq, wk, wv, wo):
    global _PROG
    if _PROG is None:
        _PROG = build_program()
    nc = _PROG

    twq = _ternarize(wq) / 8.0          # fold softmax scale into q
    twk = _ternarize(wk)
    twv = _ternarize(wv)
    two = _ternarize(wo)
    tri_np = ((1.0 - np.tril(np.ones((128, 128)))) * -1e30).astype(bf)

    in_maps = []
    for c in range(8):
        b, hq = c % 2, c // 2
        xT = np.ascontiguousarray(x[b].astype(np.float32).T)      # [DIM, S]
        xhi, xlo = _split_hi_lo(xT)
        qcols = slice(hq * 512, (hq + 1) * 512)
        kvcols = slice(hq * 128, (hq + 1) * 128)
        in_maps.append({
            "xhi": xhi,
            "xlo": xlo,
            "wq": np.ascontiguousarray(twq.T[:, qcols]).astype(bf),
            "wk": np.ascontiguousarray(twk.T[:, kvcols]).astype(bf),
            "wv": np.ascontiguousarray(twv.T[:, kvcols]).astype(bf),
            "wo": np.ascontiguousarray(two.T[hq * 512:(hq + 1) * 512, :]).astype(np.float32),
            "tri": tri_np,
        })

    res = run_bass_kernel_spmd(nc, in_maps, list(range(8)))

    out = np.zeros((BSZ, SEQ, DIM), np.float32)
    for c in range(8):
        b = c % 2
        out[b] += res.results[c]["oT"].T
    return out

